# revision 10
# baseline (speedup 1.0000x reference)
"""Trainium2 Bass kernel for nn_MoE_28157805592688.

MoE with cross-attention routing. Key identities used:
  - The reference's cross-attention softmax is over a size-1 axis, so it is
    exactly 1.0 and att = (q @ Wv + bv) @ Wo + bo. Q/K projections are unused.
  - att only feeds the gate, so gate_logits = q @ G + g0 with
    G = Wv @ Wo @ gate_W,  g0 = bv @ Wo @ gate_W + bo @ gate_W + gate_b.
  - softmax is monotonic, so top-2 selection can be done on gate_prob directly
    (reference does top_k on gate_prob; we use the prob values for the
    renormalized weights exactly as the reference does).

Distribution: data-parallel over the 8192 tokens across 8 NeuronCores (1024
tokens/core); all weights replicated per core (no collectives). The gate path
runs in full fp32 (top-2 selection is numerically fragile: min prob-gap between
2nd and 3rd expert is ~1e-6, so fp32r noise there would flip selections); the
expert MLP runs in fp32r (full PE rate, ~1.5e-4 rel rounding).
"""

import numpy as np

D = 512
E = 8
HID = 512
NTOK = 8192
NCORES = 8
T = NTOK // NCORES  # tokens per core
P = 128
KO = D // P          # 4 k-tiles of 128 over D (and over HID)
NTC = T // 512       # 512-token chunks per core (2)
NCH = T // P         # 128-token chunks per core (8)
W_IMP = 0.01

_CACHE = {}


def _build_bass(with_b2: bool):
    import concourse.bass as bass
    import concourse.mybir as mybir
    from concourse import bacc
    from concourse.tile import TileContext

    fp32 = mybir.dt.float32
    fp32r = mybir.dt.float32r
    u32 = mybir.dt.uint32

    nc = bacc.Bacc(None, target_bir_lowering=False)

    # ---- I/O ----
    # xT/W1/W2 are declared float32r: the PE reads fp32r operands from their
    # high bits, so feeding raw fp32 bytes through a plain HWDGE DMA gives the
    # same numerics as an explicit rounding pass while keeping loads off the
    # (slower, serialized) SWDGE cast path.
    xT_d = nc.dram_tensor("xT", [D, T], fp32r, kind="ExternalInput")
    qT_d = nc.dram_tensor("qT", [D, T], fp32, kind="ExternalInput")
    G_d = nc.dram_tensor("G", [D, E], fp32, kind="ExternalInput")
    g0_d = nc.dram_tensor("g0", [P, E], fp32, kind="ExternalInput")
    W1_d = nc.dram_tensor("W1", [E, D, HID], fp32r, kind="ExternalInput")
    W2_d = nc.dram_tensor("W2", [E, HID, D], fp32r, kind="ExternalInput")
    b1T_d = nc.dram_tensor("b1T", [HID, E], fp32, kind="ExternalInput")
    y_d = nc.dram_tensor("y", [T, D], fp32, kind="ExternalOutput")
    gp_d = nc.dram_tensor("gp", [T, E], fp32, kind="ExternalOutput")

    with TileContext(nc) as tc:
        with (
            tc.tile_pool(name="const", bufs=1) as cpool,
            tc.tile_pool(name="w", bufs=3) as wpool,
            tc.tile_pool(name="h", bufs=2) as hpool,
            tc.tile_pool(name="sc", bufs=4) as spool,
            tc.tile_pool(name="ps1", bufs=4, space="PSUM") as ps1,
            tc.tile_pool(name="ps2", bufs=4, space="PSUM") as ps2,
        ):
            psg = ps2
            # ---- constant loads ----
            # The first layer-1 matmul only needs the k=0 slices of xT and
            # W1[0], so those loads are split per-k and interleaved on the
            # sync queue; everything not on the first-matmul critical path
            # goes to the scalar queue.
            xTr = cpool.tile([P, KO, T], fp32r)
            xT_r = xT_d.rearrange("(ko p) t -> p ko t", p=P)
            w1t0 = wpool.tile([P, KO, HID], fp32r, tag="w1")
            w1_r0 = W1_d[0].rearrange("(ko p) h -> p ko h", p=P)
            for k in range(KO):
                nc.sync.dma_start(xTr[:, k:k + 1, :], xT_r[:, k:k + 1, :])
                nc.sync.dma_start(w1t0[:, k:k + 1, :], w1_r0[:, k:k + 1, :])
            qT = cpool.tile([P, KO, T], fp32)
            nc.scalar.dma_start(qT, qT_d.rearrange("(ko p) t -> p ko t", p=P))
            G_sb = cpool.tile([P, KO, E], fp32)
            nc.scalar.dma_start(G_sb, G_d.rearrange("(ko p) e -> p ko e", p=P))
            g0_sb = cpool.tile([P, E], fp32)
            nc.scalar.dma_start(g0_sb, g0_d[:, :])
            b1T_sb = cpool.tile([P, KO, E], fp32)
            nc.scalar.dma_start(b1T_sb, b1T_d.rearrange("(ko p) e -> p ko e", p=P))

            iotaE = cpool.tile([P, E], u32)
            nc.gpsimd.iota(iotaE, pattern=[[1, E]], base=0, channel_multiplier=0)

            gp_sb = cpool.tile([P, NCH, E], fp32)
            comb_sb = cpool.tile([P, NCH, E], fp32)
            y_sb = cpool.tile([P, NCH, D], fp32)

            w1ts = {}
            w2ts = {}

            def load_weights(e, w1_pre=None):
                if w1_pre is None:
                    w1t = wpool.tile([P, KO, HID], fp32r, tag="w1")
                    eng = nc.sync if e % 2 == 0 else nc.scalar
                    eng.dma_start(w1t, W1_d[e].rearrange("(ko p) h -> p ko h", p=P))
                else:
                    w1t = w1_pre
                    eng = nc.sync
                w2t = wpool.tile([P, KO, D], fp32r, tag="w2")
                eng.dma_start(w2t, W2_d[e].rearrange("(ko p) d -> p ko d", p=P))
                w1ts[e] = w1t
                w2ts[e] = w2t

            def emit_l1(e):
                # layer 1: h[hid, tok] = W1[e].T-block @ xT; relu(+b1) -> fp32r
                hr = hpool.tile([P, KO, T], fp32r, tag="hr")
                for hs in range(KO):
                    for tch in range(NTC):
                        h_ps = ps1.tile([P, 512], fp32, tag="ps1")
                        for k in range(KO):
                            nc.tensor.matmul(
                                h_ps,
                                w1ts[e][:, k, hs * P:(hs + 1) * P],
                                xTr[:, k, tch * 512:(tch + 1) * 512],
                                start=(k == 0),
                                stop=(k == KO - 1),
                            )
                        nc.vector.tensor_scalar(
                            hr[:, hs, tch * 512:(tch + 1) * 512],
                            h_ps,
                            b1T_sb[:, hs, e:e + 1],
                            0.0,
                            op0=mybir.AluOpType.add,
                            op1=mybir.AluOpType.max,
                        )
                return hr

            def emit_l2(e, hr, last):
                # layer 2: ye[tok, d] = hr.T-block @ W2[e]; scale by comb; y +=
                for ts in range(NCH):
                    ye_ps = ps2.tile([P, 512], fp32, tag="ps2")
                    for k in range(KO):
                        nc.tensor.matmul(
                            ye_ps,
                            hr[:, k, ts * P:(ts + 1) * P],
                            w2ts[e][:, k, :],
                            start=(k == 0),
                            stop=(k == KO - 1),
                        )
                    if e == 0:
                        nc.scalar.activation(
                            y_sb[:, ts, :], ye_ps,
                            mybir.ActivationFunctionType.Copy,
                            scale=comb_sb[:, ts, e:e + 1],
                        )
                    else:
                        ysc = spool.tile([P, 512], fp32, tag="ysc")
                        nc.scalar.activation(
                            ysc, ye_ps,
                            mybir.ActivationFunctionType.Copy,
                            scale=comb_sb[:, ts, e:e + 1],
                        )
                        nc.vector.tensor_tensor(
                            y_sb[:, ts, :], y_sb[:, ts, :], ysc,
                            mybir.AluOpType.add,
                        )
                    if last:
                        nc.sync.dma_start(
                            y_d.rearrange("(c p) d -> p c d", p=P)[:, ts, :],
                            y_sb[:, ts, :],
                        )
                del w1ts[e], w2ts[e]

            # expert-0 weights + layer-1 first so the PE warms up while the
            # gate path's DVE/ACT chain runs
            load_weights(0, w1_pre=w1t0)
            load_weights(1)
            hr0 = emit_l1(0)

            # ---- gate path (fp32) ----
            for c in range(NCH):
                lg_ps = psg.tile([P, E], fp32, tag="ps2")
                for k in range(KO):
                    nc.tensor.matmul(
                        lg_ps,
                        qT[:, k, c * P:(c + 1) * P],
                        G_sb[:, k, :],
                        start=(k == 0),
                        stop=(k == KO - 1),
                    )
                logits = spool.tile([P, E], fp32, tag="logits")
                nc.vector.tensor_tensor(logits, lg_ps, g0_sb, mybir.AluOpType.add)

                lmax = spool.tile([P, 1], fp32, tag="lmax")
                nc.vector.reduce_max(lmax, logits, axis=mybir.AxisListType.X)
                negmax = spool.tile([P, 1], fp32, tag="negmax")
                nc.vector.tensor_scalar_mul(negmax, lmax, -1.0)
                exps = spool.tile([P, E], fp32, tag="exps")
                sumexp = spool.tile([P, 1], fp32, tag="sumexp")
                nc.scalar.activation(
                    exps, logits, mybir.ActivationFunctionType.Exp,
                    bias=negmax, scale=1.0, accum_out=sumexp,
                )
                rcp = spool.tile([P, 1], fp32, tag="rcp")
                nc.vector.reciprocal(rcp, sumexp)
                nc.vector.tensor_scalar_mul(gp_sb[:, c, :], exps, rcp)

                # top-2 on gate_prob
                max8 = spool.tile([P, 8], fp32, tag="max8")
                idx8 = spool.tile([P, 8], u32, tag="idx8")
                nc.vector.max(max8, gp_sb[:, c, :])
                nc.vector.max_index(idx8, max8, gp_sb[:, c, :])

                # renormalized top-2 weights: softmax([p1, p2])
                dd = spool.tile([P, 1], fp32, tag="dd")
                nc.vector.tensor_tensor(
                    dd, max8[:, 1:2], max8[:, 0:1], mybir.AluOpType.subtract
                )
                ee = spool.tile([P, 1], fp32, tag="ee")
                nc.scalar.activation(ee, dd, mybir.ActivationFunctionType.Exp)
                ss = spool.tile([P, 1], fp32, tag="ss")
                nc.vector.tensor_scalar_add(ss, ee, 1.0)
                w1c = spool.tile([P, 1], fp32, tag="w1c")
                nc.vector.reciprocal(w1c, ss)
                w2c = spool.tile([P, 1], fp32, tag="w2c")
                nc.vector.tensor_tensor(w2c, ee, w1c, mybir.AluOpType.mult)

                # comb[:, e] = w1*(e==i1) + w2*(e==i2)
                m1 = spool.tile([P, E], fp32, tag="m1")
                nc.vector.tensor_tensor(
                    m1, iotaE, idx8[:, 0:1].to_broadcast([P, E]),
                    mybir.AluOpType.is_equal,
                )
                m2 = spool.tile([P, E], fp32, tag="m2")
                nc.vector.tensor_tensor(
                    m2, iotaE, idx8[:, 1:2].to_broadcast([P, E]),
                    mybir.AluOpType.is_equal,
                )
                nc.vector.tensor_scalar_mul(m1, m1, w1c)
                nc.vector.tensor_scalar_mul(m2, m2, w2c)
                nc.vector.tensor_tensor(
                    comb_sb[:, c, :], m1, m2, mybir.AluOpType.add
                )

            nc.scalar.dma_start(gp_d.rearrange("(c p) e -> p c e", p=P), gp_sb)

            # ---- expert MLP (fp32r), dense over all experts, software
            # pipelined: L1(e+1) is emitted before L2(e) so the PE never
            # waits on the hr eviction chain ----
            emit_l2(0, hr0, last=False)
            hr_prev = None
            for e in range(1, E):
                if e + 1 < E:
                    load_weights(e + 1)
                hr_prev = emit_l1(e)
                emit_l2(e, hr_prev, last=(e == E - 1))

    nc.compile()
    return nc


def kernel(**inputs):
    from concourse import bass_utils

    x = np.ascontiguousarray(np.asarray(inputs["x"], dtype=np.float32))
    q = np.ascontiguousarray(np.asarray(inputs["q"], dtype=np.float32))
    Wv = np.asarray(inputs["Wv"], dtype=np.float32)
    Wo = np.asarray(inputs["Wo"], dtype=np.float32)
    bv = np.asarray(inputs["bv"], dtype=np.float32)
    bo = np.asarray(inputs["bo"], dtype=np.float32)
    gate_W = np.asarray(inputs["gate_W"], dtype=np.float32)
    gate_b = np.asarray(inputs["gate_b"], dtype=np.float32)
    W1 = np.ascontiguousarray(np.asarray(inputs["W1"], dtype=np.float32))
    W2 = np.ascontiguousarray(np.asarray(inputs["W2"], dtype=np.float32))
    b1 = np.asarray(inputs["b1"], dtype=np.float32)
    b2 = np.asarray(inputs["b2"], dtype=np.float32)
    top = int(inputs["top"])
    assert top == 2, f"kernel hardcodes top=2, got {top}"

    x_shape = x.shape
    xf = x.reshape(-1, x_shape[-1])  # [NTOK, D]
    assert xf.shape == (NTOK, D) and q.shape == (NTOK, D)

    # fused gate projection (fp64 for accuracy, then fp32)
    G = (Wv.astype(np.float64) @ Wo.astype(np.float64) @ gate_W.astype(np.float64))
    g0 = (
        bv.astype(np.float64) @ Wo.astype(np.float64) @ gate_W.astype(np.float64)
        + bo.astype(np.float64) @ gate_W.astype(np.float64)
        + gate_b.astype(np.float64)
    )
    G = np.ascontiguousarray(G.astype(np.float32))
    g0b = np.ascontiguousarray(
        np.broadcast_to(g0.astype(np.float32), (P, E)).copy()
    )
    b1T = np.ascontiguousarray(b1.T)  # [HID, E]

    with_b2 = bool(np.any(b2))
    key = ("dense", with_b2)
    if key not in _CACHE:
        _CACHE[key] = _build_bass(with_b2)
    nc = _CACHE[key]

    in_maps = []
    for c in range(NCORES):
        sl = slice(c * T, (c + 1) * T)
        in_maps.append({
            "xT": np.ascontiguousarray(xf[sl].T),
            "qT": np.ascontiguousarray(q[sl].T),
            "G": G,
            "g0": g0b,
            "W1": W1,
            "W2": W2,
            "b1T": b1T,
        })

    res = bass_utils.run_bass_kernel_spmd(nc, in_maps, core_ids=list(range(NCORES)))

    y = np.concatenate([r["y"] for r in res.results], axis=0)
    gate_prob = np.concatenate([r["gp"] for r in res.results], axis=0)

    if with_b2:
        # b2 contribution: y += comb @ b2 (host; b2 is zero in the shipped
        # problem so this path is normally dead)
        sp = np.sort(gate_prob, axis=1)[:, ::-1]
        ti = np.argsort(-gate_prob, axis=1, kind="stable")[:, :2]
        tw = sp[:, :2]
        twe = np.exp(tw - tw[:, :1])
        twn = twe / twe.sum(axis=1, keepdims=True)
        comb = np.zeros((NTOK, E), np.float32)
        np.put_along_axis(comb, ti, twn.astype(np.float32), axis=1)
        y = y + comb @ b2

    importance = gate_prob.astype(np.float64).sum(axis=0)
    std = importance.std(ddof=1)
    mean = importance.mean()
    importance_loss = np.float32(W_IMP * (std / mean) ** 2)

    return y.reshape(x_shape), gate_prob, importance_loss


def timed_run(inputs, stitch=False):
    """Test-only helper: run once with NTFF tracing, return exec_time_ns."""
    from concourse import bass_utils

    x = np.asarray(inputs["x"], dtype=np.float32)
    q = np.asarray(inputs["q"], dtype=np.float32)
    xf = x.reshape(-1, D)
    Wv = np.asarray(inputs["Wv"], dtype=np.float32)
    Wo = np.asarray(inputs["Wo"], dtype=np.float32)
    gate_W = np.asarray(inputs["gate_W"], dtype=np.float32)
    G = np.ascontiguousarray(
        (Wv.astype(np.float64) @ Wo.astype(np.float64) @ gate_W.astype(np.float64)).astype(np.float32)
    )
    g0b = np.zeros((P, E), np.float32)
    b1T = np.ascontiguousarray(np.asarray(inputs["b1"], dtype=np.float32).T)
    key = ("dense", False)
    if key not in _CACHE:
        _CACHE[key] = _build_bass(False)
    nc = _CACHE[key]
    in_maps = []
    for c in range(NCORES):
        sl = slice(c * T, (c + 1) * T)
        in_maps.append({
            "xT": np.ascontiguousarray(xf[sl].T),
            "qT": np.ascontiguousarray(q[sl].T),
            "G": G,
            "g0": g0b,
            "W1": np.ascontiguousarray(np.asarray(inputs["W1"], dtype=np.float32)),
            "W2": np.ascontiguousarray(np.asarray(inputs["W2"], dtype=np.float32)),
            "b1T": b1T,
        })
    res = bass_utils.run_bass_kernel_spmd(
        nc, in_maps, core_ids=list(range(NCORES)), trace=True,
        trace_cores=list(range(NCORES)) if stitch else None,
        stitch_traces=stitch,
    )
    if res.instructions_and_trace is not None:
        print("trace:", res.instructions_and_trace[1])
    print("mean exec:", res.mean_exec_time_ns, "max core:", res.max_exec_time_core_id)
    return res.exec_time_ns


# revision 11
# speedup vs baseline: 1.0191x; 1.0191x over previous
"""Trainium2 Bass kernel for nn_MoE_28157805592688.

MoE with cross-attention routing. Key identities used:
  - The reference's cross-attention softmax is over a size-1 axis, so it is
    exactly 1.0 and att = (q @ Wv + bv) @ Wo + bo. Q/K projections are unused.
  - att only feeds the gate, so gate_logits = q @ G + g0 with
    G = Wv @ Wo @ gate_W,  g0 = bv @ Wo @ gate_W + bo @ gate_W + gate_b.
  - softmax is monotonic, so top-2 selection can be done on gate_prob directly
    (reference does top_k on gate_prob; we use the prob values for the
    renormalized weights exactly as the reference does).

Distribution: data-parallel over the 8192 tokens across 8 NeuronCores (1024
tokens/core); all weights replicated per core (no collectives). The gate path
runs in full fp32 (top-2 selection is numerically fragile: min prob-gap between
2nd and 3rd expert is ~1e-6, so fp32r noise there would flip selections); the
expert MLP runs in fp32r (full PE rate, ~1.5e-4 rel rounding).
"""

import numpy as np

D = 512
E = 8
HID = 512
NTOK = 8192
NCORES = 8
T = NTOK // NCORES  # tokens per core
P = 128
KO = D // P          # 4 k-tiles of 128 over D (and over HID)
NTC = T // 512       # 512-token chunks per core (2)
NCH = T // P         # 128-token chunks per core (8)
W_IMP = 0.01

_CACHE = {}


def _build_bass(with_b2: bool):
    import concourse.bass as bass
    import concourse.mybir as mybir
    from concourse import bacc
    from concourse.tile import TileContext

    fp32 = mybir.dt.float32
    fp32r = mybir.dt.float32r
    u32 = mybir.dt.uint32

    nc = bacc.Bacc(None, target_bir_lowering=False)

    # ---- I/O ----
    # xT/W1/W2 are declared float32r: the PE reads fp32r operands from their
    # high bits, so feeding raw fp32 bytes through a plain HWDGE DMA gives the
    # same numerics as an explicit rounding pass while keeping loads off the
    # (slower, serialized) SWDGE cast path.
    xT_d = nc.dram_tensor("xT", [D, T], fp32r, kind="ExternalInput")
    qT_d = nc.dram_tensor("qT", [D, T], fp32, kind="ExternalInput")
    G_d = nc.dram_tensor("G", [D, E], fp32, kind="ExternalInput")
    g0_d = nc.dram_tensor("g0", [P, E], fp32, kind="ExternalInput")
    W1_d = nc.dram_tensor("W1", [E, D, HID], fp32r, kind="ExternalInput")
    W2_d = nc.dram_tensor("W2", [E, HID, D], fp32r, kind="ExternalInput")
    b1T_d = nc.dram_tensor("b1T", [HID, E], fp32, kind="ExternalInput")
    y_d = nc.dram_tensor("y", [T, D], fp32, kind="ExternalOutput")
    gp_d = nc.dram_tensor("gp", [T, E], fp32, kind="ExternalOutput")

    with TileContext(nc) as tc:
        with (
            tc.tile_pool(name="const", bufs=1) as cpool,
            tc.tile_pool(name="w", bufs=4) as wpool,
            tc.tile_pool(name="h", bufs=3) as hpool,
            tc.tile_pool(name="sc", bufs=4) as spool,
            tc.tile_pool(name="ps1", bufs=4, space="PSUM") as ps1,
            tc.tile_pool(name="ps2", bufs=4, space="PSUM") as ps2,
        ):
            psg = ps2
            # ---- constant loads ----
            # The first layer-1 matmul only needs the k=0 slices of xT and
            # W1[0], so those loads are split per-k and interleaved on the
            # sync queue; everything not on the first-matmul critical path
            # goes to the scalar queue.
            xTr = cpool.tile([P, KO, T], fp32r)
            xT_r = xT_d.rearrange("(ko p) t -> p ko t", p=P)
            w1t0 = wpool.tile([P, KO, HID], fp32r, tag="w1")
            w1_r0 = W1_d[0].rearrange("(ko p) h -> p ko h", p=P)
            for k in range(KO):
                nc.sync.dma_start(xTr[:, k:k + 1, :], xT_r[:, k:k + 1, :])
                nc.sync.dma_start(w1t0[:, k:k + 1, :], w1_r0[:, k:k + 1, :])
            qT = cpool.tile([P, KO, T], fp32)
            nc.scalar.dma_start(qT, qT_d.rearrange("(ko p) t -> p ko t", p=P))
            G_sb = cpool.tile([P, KO, E], fp32)
            nc.scalar.dma_start(G_sb, G_d.rearrange("(ko p) e -> p ko e", p=P))
            g0_sb = cpool.tile([P, E], fp32)
            nc.scalar.dma_start(g0_sb, g0_d[:, :])
            b1T_sb = cpool.tile([P, KO, E], fp32)
            nc.scalar.dma_start(b1T_sb, b1T_d.rearrange("(ko p) e -> p ko e", p=P))

            iotaE = cpool.tile([P, E], u32)
            nc.gpsimd.iota(iotaE, pattern=[[1, E]], base=0, channel_multiplier=0)

            gp_sb = cpool.tile([P, NCH, E], fp32)
            comb_sb = cpool.tile([P, NCH, E], fp32)
            y_sb = cpool.tile([P, NCH, D], fp32)

            w1ts = {}
            w2ts = {}

            def load_weights(e, w1_pre=None):
                if w1_pre is None:
                    w1t = wpool.tile([P, KO, HID], fp32r, tag="w1")
                    eng = nc.sync if e % 2 == 0 else nc.scalar
                    eng.dma_start(w1t, W1_d[e].rearrange("(ko p) h -> p ko h", p=P))
                else:
                    w1t = w1_pre
                    eng = nc.sync
                w2t = wpool.tile([P, KO, D], fp32r, tag="w2")
                eng.dma_start(w2t, W2_d[e].rearrange("(ko p) d -> p ko d", p=P))
                w1ts[e] = w1t
                w2ts[e] = w2t

            def emit_l1(e):
                # layer 1: h[hid, tok] = W1[e].T-block @ xT; relu(+b1) -> fp32r
                hr = hpool.tile([P, KO, T], fp32r, tag="hr")
                for hs in range(KO):
                    for tch in range(NTC):
                        h_ps = ps1.tile([P, 512], fp32, tag="ps1")
                        for k in range(KO):
                            nc.tensor.matmul(
                                h_ps,
                                w1ts[e][:, k, hs * P:(hs + 1) * P],
                                xTr[:, k, tch * 512:(tch + 1) * 512],
                                start=(k == 0),
                                stop=(k == KO - 1),
                            )
                        nc.vector.tensor_scalar(
                            hr[:, hs, tch * 512:(tch + 1) * 512],
                            h_ps,
                            b1T_sb[:, hs, e:e + 1],
                            0.0,
                            op0=mybir.AluOpType.add,
                            op1=mybir.AluOpType.max,
                        )
                return hr

            def emit_l2(e, hr, last):
                # layer 2: ye[tok, d] = hr.T-block @ W2[e]; scale by comb; y +=
                for ts in range(NCH):
                    ye_ps = ps2.tile([P, 512], fp32, tag="ps2")
                    for k in range(KO):
                        nc.tensor.matmul(
                            ye_ps,
                            hr[:, k, ts * P:(ts + 1) * P],
                            w2ts[e][:, k, :],
                            start=(k == 0),
                            stop=(k == KO - 1),
                        )
                    if e == 0:
                        nc.scalar.activation(
                            y_sb[:, ts, :], ye_ps,
                            mybir.ActivationFunctionType.Copy,
                            scale=comb_sb[:, ts, e:e + 1],
                        )
                    else:
                        ysc = spool.tile([P, 512], fp32, tag="ysc")
                        nc.scalar.activation(
                            ysc, ye_ps,
                            mybir.ActivationFunctionType.Copy,
                            scale=comb_sb[:, ts, e:e + 1],
                        )
                        nc.vector.tensor_tensor(
                            y_sb[:, ts, :], y_sb[:, ts, :], ysc,
                            mybir.AluOpType.add,
                        )
                    if last:
                        nc.sync.dma_start(
                            y_d.rearrange("(c p) d -> p c d", p=P)[:, ts, :],
                            y_sb[:, ts, :],
                        )
                del w1ts[e], w2ts[e]

            # expert-0 weights + layer-1 first so the PE warms up while the
            # gate path's DVE/ACT chain runs
            load_weights(0, w1_pre=w1t0)
            load_weights(1)
            hr0 = emit_l1(0)

            # ---- gate path (fp32) ----
            for c in range(NCH):
                lg_ps = psg.tile([P, E], fp32, tag="ps2")
                for k in range(KO):
                    nc.tensor.matmul(
                        lg_ps,
                        qT[:, k, c * P:(c + 1) * P],
                        G_sb[:, k, :],
                        start=(k == 0),
                        stop=(k == KO - 1),
                    )
                logits = spool.tile([P, E], fp32, tag="logits")
                nc.vector.tensor_tensor(logits, lg_ps, g0_sb, mybir.AluOpType.add)

                lmax = spool.tile([P, 1], fp32, tag="lmax")
                nc.vector.reduce_max(lmax, logits, axis=mybir.AxisListType.X)
                negmax = spool.tile([P, 1], fp32, tag="negmax")
                nc.vector.tensor_scalar_mul(negmax, lmax, -1.0)
                exps = spool.tile([P, E], fp32, tag="exps")
                sumexp = spool.tile([P, 1], fp32, tag="sumexp")
                nc.scalar.activation(
                    exps, logits, mybir.ActivationFunctionType.Exp,
                    bias=negmax, scale=1.0, accum_out=sumexp,
                )
                rcp = spool.tile([P, 1], fp32, tag="rcp")
                nc.vector.reciprocal(rcp, sumexp)
                nc.vector.tensor_scalar_mul(gp_sb[:, c, :], exps, rcp)

                # top-2 on gate_prob
                max8 = spool.tile([P, 8], fp32, tag="max8")
                idx8 = spool.tile([P, 8], u32, tag="idx8")
                nc.vector.max(max8, gp_sb[:, c, :])
                nc.vector.max_index(idx8, max8, gp_sb[:, c, :])

                # renormalized top-2 weights: softmax([p1, p2])
                dd = spool.tile([P, 1], fp32, tag="dd")
                nc.vector.tensor_tensor(
                    dd, max8[:, 1:2], max8[:, 0:1], mybir.AluOpType.subtract
                )
                ee = spool.tile([P, 1], fp32, tag="ee")
                nc.scalar.activation(ee, dd, mybir.ActivationFunctionType.Exp)
                ss = spool.tile([P, 1], fp32, tag="ss")
                nc.vector.tensor_scalar_add(ss, ee, 1.0)
                w1c = spool.tile([P, 1], fp32, tag="w1c")
                nc.vector.reciprocal(w1c, ss)
                w2c = spool.tile([P, 1], fp32, tag="w2c")
                nc.vector.tensor_tensor(w2c, ee, w1c, mybir.AluOpType.mult)

                # comb[:, e] = w1*(e==i1) + w2*(e==i2)
                m1 = spool.tile([P, E], fp32, tag="m1")
                nc.vector.tensor_tensor(
                    m1, iotaE, idx8[:, 0:1].to_broadcast([P, E]),
                    mybir.AluOpType.is_equal,
                )
                m2 = spool.tile([P, E], fp32, tag="m2")
                nc.vector.tensor_tensor(
                    m2, iotaE, idx8[:, 1:2].to_broadcast([P, E]),
                    mybir.AluOpType.is_equal,
                )
                nc.vector.tensor_scalar_mul(m1, m1, w1c)
                nc.vector.tensor_scalar_mul(m2, m2, w2c)
                nc.vector.tensor_tensor(
                    comb_sb[:, c, :], m1, m2, mybir.AluOpType.add
                )

            nc.scalar.dma_start(gp_d.rearrange("(c p) e -> p c e", p=P), gp_sb)

            # ---- expert MLP (fp32r), dense over all experts, software
            # pipelined: L1(e+1) is emitted before L2(e) so the PE never
            # waits on the hr eviction chain ----
            emit_l2(0, hr0, last=False)
            hr_prev = None
            for e in range(1, E):
                if e + 1 < E:
                    load_weights(e + 1)
                hr_prev = emit_l1(e)
                emit_l2(e, hr_prev, last=(e == E - 1))

    nc.compile()
    return nc


def kernel(**inputs):
    from concourse import bass_utils

    x = np.ascontiguousarray(np.asarray(inputs["x"], dtype=np.float32))
    q = np.ascontiguousarray(np.asarray(inputs["q"], dtype=np.float32))
    Wv = np.asarray(inputs["Wv"], dtype=np.float32)
    Wo = np.asarray(inputs["Wo"], dtype=np.float32)
    bv = np.asarray(inputs["bv"], dtype=np.float32)
    bo = np.asarray(inputs["bo"], dtype=np.float32)
    gate_W = np.asarray(inputs["gate_W"], dtype=np.float32)
    gate_b = np.asarray(inputs["gate_b"], dtype=np.float32)
    W1 = np.ascontiguousarray(np.asarray(inputs["W1"], dtype=np.float32))
    W2 = np.ascontiguousarray(np.asarray(inputs["W2"], dtype=np.float32))
    b1 = np.asarray(inputs["b1"], dtype=np.float32)
    b2 = np.asarray(inputs["b2"], dtype=np.float32)
    top = int(inputs["top"])
    assert top == 2, f"kernel hardcodes top=2, got {top}"

    x_shape = x.shape
    xf = x.reshape(-1, x_shape[-1])  # [NTOK, D]
    assert xf.shape == (NTOK, D) and q.shape == (NTOK, D)

    # fused gate projection (fp64 for accuracy, then fp32)
    G = (Wv.astype(np.float64) @ Wo.astype(np.float64) @ gate_W.astype(np.float64))
    g0 = (
        bv.astype(np.float64) @ Wo.astype(np.float64) @ gate_W.astype(np.float64)
        + bo.astype(np.float64) @ gate_W.astype(np.float64)
        + gate_b.astype(np.float64)
    )
    G = np.ascontiguousarray(G.astype(np.float32))
    g0b = np.ascontiguousarray(
        np.broadcast_to(g0.astype(np.float32), (P, E)).copy()
    )
    b1T = np.ascontiguousarray(b1.T)  # [HID, E]

    with_b2 = bool(np.any(b2))
    key = ("dense", with_b2)
    if key not in _CACHE:
        _CACHE[key] = _build_bass(with_b2)
    nc = _CACHE[key]

    in_maps = []
    for c in range(NCORES):
        sl = slice(c * T, (c + 1) * T)
        in_maps.append({
            "xT": np.ascontiguousarray(xf[sl].T),
            "qT": np.ascontiguousarray(q[sl].T),
            "G": G,
            "g0": g0b,
            "W1": W1,
            "W2": W2,
            "b1T": b1T,
        })

    res = bass_utils.run_bass_kernel_spmd(nc, in_maps, core_ids=list(range(NCORES)))

    y = np.concatenate([r["y"] for r in res.results], axis=0)
    gate_prob = np.concatenate([r["gp"] for r in res.results], axis=0)

    if with_b2:
        # b2 contribution: y += comb @ b2 (host; b2 is zero in the shipped
        # problem so this path is normally dead)
        sp = np.sort(gate_prob, axis=1)[:, ::-1]
        ti = np.argsort(-gate_prob, axis=1, kind="stable")[:, :2]
        tw = sp[:, :2]
        twe = np.exp(tw - tw[:, :1])
        twn = twe / twe.sum(axis=1, keepdims=True)
        comb = np.zeros((NTOK, E), np.float32)
        np.put_along_axis(comb, ti, twn.astype(np.float32), axis=1)
        y = y + comb @ b2

    importance = gate_prob.astype(np.float64).sum(axis=0)
    std = importance.std(ddof=1)
    mean = importance.mean()
    importance_loss = np.float32(W_IMP * (std / mean) ** 2)

    return y.reshape(x_shape), gate_prob, importance_loss


def timed_run(inputs, stitch=False):
    """Test-only helper: run once with NTFF tracing, return exec_time_ns."""
    from concourse import bass_utils

    x = np.asarray(inputs["x"], dtype=np.float32)
    q = np.asarray(inputs["q"], dtype=np.float32)
    xf = x.reshape(-1, D)
    Wv = np.asarray(inputs["Wv"], dtype=np.float32)
    Wo = np.asarray(inputs["Wo"], dtype=np.float32)
    gate_W = np.asarray(inputs["gate_W"], dtype=np.float32)
    G = np.ascontiguousarray(
        (Wv.astype(np.float64) @ Wo.astype(np.float64) @ gate_W.astype(np.float64)).astype(np.float32)
    )
    g0b = np.zeros((P, E), np.float32)
    b1T = np.ascontiguousarray(np.asarray(inputs["b1"], dtype=np.float32).T)
    key = ("dense", False)
    if key not in _CACHE:
        _CACHE[key] = _build_bass(False)
    nc = _CACHE[key]
    in_maps = []
    for c in range(NCORES):
        sl = slice(c * T, (c + 1) * T)
        in_maps.append({
            "xT": np.ascontiguousarray(xf[sl].T),
            "qT": np.ascontiguousarray(q[sl].T),
            "G": G,
            "g0": g0b,
            "W1": np.ascontiguousarray(np.asarray(inputs["W1"], dtype=np.float32)),
            "W2": np.ascontiguousarray(np.asarray(inputs["W2"], dtype=np.float32)),
            "b1T": b1T,
        })
    res = bass_utils.run_bass_kernel_spmd(
        nc, in_maps, core_ids=list(range(NCORES)), trace=True,
        trace_cores=list(range(NCORES)) if stitch else None,
        stitch_traces=stitch,
    )
    if res.instructions_and_trace is not None:
        print("trace:", res.instructions_and_trace[1])
    print("mean exec:", res.mean_exec_time_ns, "max core:", res.max_exec_time_core_id)
    return res.exec_time_ns
